# revision 15
# baseline (speedup 1.0000x reference)
"""Trainium2 Bass kernel for nn_Compression.

Computes: out = X + GAMMA * (P @ (P.T @ X)),  P = softmax(X @ W.T + b)

Strategy (8 NeuronCores, data-parallel over N):
  - Each core owns NLOC = N/8 = 4096 rows of X (32 tiles of 128 rows).
  - X ships to the device as bf16 (host-side cast, pure relayout of the
    shard): the residual add only needs bf16 precision (~1.7e-3
    relative, vs the 2e-2 gate) and this halves the input DMA.
  - fp8e4 in the correction path: the correction term is ~1e-5 of the
    output magnitude, so fp8 quantization (~6% of the correction)
    contributes ~1e-6 relative.  fp8 enables DoubleRow matmuls (two
    contraction rows per cycle) which halve the logits / corr PE time.
    Transposes stay bf16 (fp8 PE transposes need strided outputs) and
    cast to fp8 at the PSUM drain.
  - Phase A per row-tile: PE-transpose of the bf16 tile (drained to
    fp8), logits via 4 DoubleRow matmuls over d-chunk pairs against
    16*W in fp8 (the 1/16 un-scale rides the exp activation's scale),
    softmax on ScalarE.  P.T @ X accumulates into 4 resident PSUM
    banks in bf16 (an fp8 X copy costs more in casts than DoubleRow
    saves on the PE).
  - The PtX accumulation is split into two row-groups sharing the same
    PSUM banks.  Group A's partial is drained to fp8 and AllReduced
    *while phase A keeps running* on the second group, hiding that
    collective (the early SPLIT keeps it clear of the second trigger,
    since collectives serialize on one stream); only group B's
    AllReduce (256 KiB fp8) is exposed at the end, and filler PE
    transposes reading the group-B stage (so the scheduler cannot
    hoist them into phase A) keep the HAM clock-gate warm across it.
  - Phase B: ptxb = arA + arB (fp8), corr = P @ ptxb as one DoubleRow
    matmul per D-half, then out = GAMMA*corr + X as a fused
    scalar_tensor_tensor on DVE / GpSimd, one 512 KiB DMA per tile.
"""

import sys

import numpy as np

if "/opt/trn_rl_repo" not in sys.path:
    sys.path.insert(0, "/opt/trn_rl_repo")

N, D, C = 32768, 1024, 256
GAMMA = 1e-4
WSCALE = 16.0  # keeps 16*W inside fp8e4's normal range
NCORES = 8
NLOC = N // NCORES  # 4096
P = 128
NT = NLOC // P  # 32
DH = 512
NPAIR = NT // 2  # 16
PSPLIT = 5  # row-tile pairs in AllReduce group A (triggered early enough
            # that the contended ~36us collective clears phase A's end, so
            # the exposed group-B AllReduce isn't stream-serialized)
NFILL = 600  # PE warm-keeper transposes across the exposed AllReduce

_cache = {}


def _build_nc():
    import concourse.tile as tile
    from concourse import bacc
    import concourse.mybir as mybir
    from concourse.masks import make_identity
    from contextlib import ExitStack

    f32 = mybir.dt.float32
    bf16 = mybir.dt.bfloat16
    f8 = mybir.dt.float8e4
    AF = mybir.ActivationFunctionType
    DR = mybir.MatmulPerfMode.DoubleRow

    nc = bacc.Bacc("TRN2", target_bir_lowering=False, debug=False, num_devices=NCORES)
    Xbf = nc.dram_tensor("Xbf", [NLOC, D], bf16, kind="ExternalInput").ap()
    Wt = nc.dram_tensor("Wt", [D, C], f32, kind="ExternalInput").ap()
    bvec = nc.dram_tensor("b", [C], f32, kind="ExternalInput").ap()
    out = nc.dram_tensor("out", [NLOC, D], f32, kind="ExternalOutput").ap()

    with tile.TileContext(nc) as tc, ExitStack() as ctx:
        const = ctx.enter_context(tc.tile_pool(name="const", bufs=1))
        xres = ctx.enter_context(tc.tile_pool(name="xres", bufs=1))
        work = ctx.enter_context(tc.tile_pool(name="work", bufs=2))
        ppool = ctx.enter_context(tc.tile_pool(name="ppool", bufs=4))
        spool = ctx.enter_context(tc.tile_pool(name="spool", bufs=4))
        stgp = ctx.enter_context(tc.tile_pool(name="stgp", bufs=2))
        opool = ctx.enter_context(tc.tile_pool(name="opool", bufs=4))
        cpool = ctx.enter_context(tc.tile_pool(name="cpool", bufs=4))
        dram = ctx.enter_context(tc.tile_pool(name="dram", bufs=1, space="DRAM"))

        Xall = xres.tile([P, NT, D], bf16)  # residual + transpose operand
        Xf8 = xres.tile([P, NT, D], f8)  # PtX matmul operand (DVE cast)
        Pt = const.tile([P, 2, NLOC], f8)  # P.T resident (unscaled)
        # row-tile-pair P buffers for the DoubleRow PtX ([n, tile-in-pair, c])
        p8buf = [const.tile([P, 2, C], f8, name=f"p8_{k}") for k in range(3)]

        # X tiles 0-2 first so tile 0's transpose isn't behind the W-chunk
        # DMAs in the queue.
        def s_load(i):
            nc.sync.dma_start(Xall[:, i, :], Xbf[i * P:(i + 1) * P, :])
            nc.vector.tensor_copy(Xf8[:, i, :], Xall[:, i, :])

        ident = const.tile([P, P], bf16)
        make_identity(nc, ident)

        # 16*W.T in fp8, [d-within-chunk, k-chunk, c]; 4 DMA chunks (the
        # first two ahead of the X prefetch so logits(0) isn't starved),
        # cast on ScalarE.
        Wt8 = const.tile([P, 8, C], f8)
        wtmp = ctx.enter_context(tc.tile_pool(name="wtmp", bufs=1))
        wt_f = wtmp.tile([P, 8, C], f32)
        wt_r = Wt.rearrange("(k p) c -> p k c", p=P)

        def load_w(q):
            nc.sync.dma_start(wt_f[:, 2 * q:2 * q + 2, :], wt_r[:, 2 * q:2 * q + 2, :])
            nc.scalar.mul(Wt8[:, 2 * q:2 * q + 2, :], wt_f[:, 2 * q:2 * q + 2, :], WSCALE)

        load_w(0)
        load_w(1)
        for i in range(3):
            s_load(i)
        load_w(2)
        load_w(3)

        ones1 = const.tile([1, P], bf16)
        nc.vector.memset(ones1[:], 1.0)
        b_sb = const.tile([1, C], bf16)  # 16*b
        with tc.tile_pool(name="btmp", bufs=1) as btmp:
            b_f = btmp.tile([1, C], f32)
            nc.sync.dma_start(b_f[:], bvec.rearrange("(o c) -> o c", o=1))
            nc.vector.tensor_scalar_mul(b_sb[:], b_f[:], WSCALE)

        # Two AllReduces (fp8e4, unscaled partials): group A hidden under
        # phase A's second half, group B exposed at the end.
        ar_in = [dram.tile([C, D], f8, name=f"ar_in{g}") for g in range(2)]
        ar_out = [
            dram.tile([C, D], f8, addr_space="Shared", name=f"ar_out{g}")
            for g in range(2)
        ]

        # ---- phase A: software-pipelined over row-tiles ----
        def s_transpose(i):
            xt = work.tile([P, D], f8, name="xt", tag="xt")
            trp = psA.tile([P, D], bf16, name="trp", tag="trp")
            for k in range(8):
                nc.tensor.matmul(
                    trp[:, k * P:(k + 1) * P],
                    Xall[:, i, k * P:(k + 1) * P],
                    ident[:],
                    is_transpose=True,
                    start=(k == 0),
                    stop=(k == 7),
                )
            # drain + fp8 cast, split across ACT and DVE
            nc.scalar.copy(xt[:, 0:DH], trp[:, 0:DH])
            nc.vector.tensor_copy(xt[:, DH:D], trp[:, DH:D])
            return xt

        def s_logits(i, xt):
            lg = psL.tile([P, C], f32, name="lg", tag="lg")
            xt_r = xt[:].rearrange("p (k n) -> p k n", k=8)
            for kk in range(4):
                nc.tensor.matmul(
                    lg[:],
                    xt_r[:, 2 * kk:2 * kk + 2, :],
                    Wt8[:, 2 * kk:2 * kk + 2, :],
                    perf_mode=DR,
                    start=(kk == 0),
                    stop=False,
                )
            nc.tensor.matmul(lg[:], ones1[:], b_sb[:], start=False, stop=True)
            return lg

        def s_softmax(i, lg):
            # logits arrive scaled by 16 (W,b pre-scaled); the 1/16 rides
            # the exp activation's scale.  |logits| <= ~10 so exp is safe
            # without max-subtraction.
            p_sb = ppool.tile([P, C], f32, name="p_sb", tag="p")
            ssum = spool.tile([P, 1], f32, name="ssum", tag="s")
            nc.scalar.activation(p_sb[:], lg[:], AF.Exp, scale=1.0 / WSCALE,
                                 accum_out=ssum[:])
            rinv = spool.tile([P, 1], f32, name="rinv", tag="r")
            nc.vector.reciprocal(rinv[:], ssum[:])
            # normalized P in bf16 (feeds both the PtX matmul and the P.T
            # transpose); fp8 would need an extra cast engine pass and the
            # DoubleRow PtX gain is eaten by it.
            p_bf = ppool.tile([P, C], bf16, name="p_bf", tag="pb")
            nc.scalar.activation(p_bf[:], p_sb[:], AF.Copy, scale=rinv[:])
            nc.scalar.activation(p8buf[(i // 2) % 3][:, i % 2, :], p_sb[:],
                                 AF.Copy, scale=rinv[:])
            return p_bf

        def s_ptx_pair(j, pbfs):
            # PtX += P_pair.T @ X_pair: fp8 DoubleRow, the pair of row-tiles
            # being the doubled k-tile (same natural [p, 2, free] layout the
            # logits DoubleRow already validated on hardware)
            pp = p8buf[j % 3]
            first = j in (0, PSPLIT)
            last = j in (PSPLIT - 1, NPAIR - 1)
            for c in range(2):
                for h in range(2):
                    nc.tensor.matmul(
                        ptx_ps[2 * c + h][:],
                        pp[:, :, c * P:(c + 1) * P],
                        Xf8[:, 2 * j:2 * j + 2, h * DH:(h + 1) * DH],
                        perf_mode=DR,
                        start=first,
                        stop=last,
                    )
            # P.T for phase B (bf16 transpose, fp8 cast at the DVE drain)
            for t, p_bf in zip((2 * j, 2 * j + 1), pbfs):
                ptp = psA.tile([P, C], bf16, name="ptp", tag="trp")
                for c in range(2):
                    nc.tensor.matmul(
                        ptp[:, c * P:(c + 1) * P],
                        p_bf[:, c * P:(c + 1) * P],
                        ident[:],
                        is_transpose=True,
                        start=(c == 0),
                        stop=(c == 1),
                    )
                nc.vector.tensor_copy(
                    Pt[:, :, t * P:(t + 1) * P],
                    ptp[:].rearrange("p (c n) -> p c n", c=2),
                )

        def drain_and_reduce(g):
            # PSUM -> SBUF fp8 casts (split across ACT and DVE), each
            # quadrant DMA'd to DRAM as soon as its cast lands, then the
            # AllReduce trigger.
            stg = stgp.tile([P, 2, D], f8, name=f"stg{g}", tag="stg")
            ar_v = ar_in[g].rearrange("(c p) d -> p c d", p=P)
            for c in range(2):
                for h in range(2):
                    dst = stg[:, c, h * DH:(h + 1) * DH]
                    src = ptx_ps[2 * c + h]
                    if h == 0:
                        nc.scalar.copy(dst, src[:])
                    else:
                        nc.vector.tensor_copy(dst, src[:])
                    nc.sync.dma_start(ar_v[:, c, h * DH:(h + 1) * DH], dst)
            nc.gpsimd.collective_compute(
                "AllReduce",
                mybir.AluOpType.add,
                replica_groups=[list(range(NCORES))],
                ins=[ar_in[g][:].opt()],
                outs=[ar_out[g][:].opt()],
            )
            return stg

        with tc.tile_pool(name="psA", bufs=3, space="PSUM") as psA, \
             tc.tile_pool(name="psL", bufs=1, space="PSUM") as psL, \
             tc.tile_pool(name="psX", bufs=1, space="PSUM") as psX:
            ptx_ps = [
                psX.tile([P, DH], f32, name=f"ptx_{c}_{h}", tag=f"ptx_{c}_{h}")
                for c in range(2)
                for h in range(2)
            ]
            # 2-3 tile skew between softmax(i) and the pair PtX: the ~1us
            # ScalarE exp latency hides under transposes + the next logits
            # block instead of stalling the PE.
            xts = {0: s_transpose(0)}
            pbf = {}
            for i in range(NT):
                lg = s_logits(i, xts.pop(i))
                pbf[i] = s_softmax(i, lg)
                if i >= 3 and i % 2 == 1:
                    j = (i - 3) // 2
                    s_ptx_pair(j, (pbf.pop(2 * j), pbf.pop(2 * j + 1)))
                    if j == PSPLIT - 1:
                        drain_and_reduce(0)
                if i + 1 < NT:
                    xts[i + 1] = s_transpose(i + 1)
                if i + 2 < NT:
                    s_load(i + 2)
            s_ptx_pair(NPAIR - 1, (pbf.pop(NT - 2), pbf.pop(NT - 1)))
            stg1 = drain_and_reduce(1)

        # ---- exposed-collective window + phase B ----
        # Keep the HAM clock-gate warm while AllReduce B flies.  The
        # fillers read the group-B stage so the scheduler cannot hoist
        # them before the end of phase A.
        with tc.tile_pool(name="psF", bufs=1, space="PSUM") as psF:
            ftile = psF.tile([P, P], bf16, name="fill", tag="fill")
            stgb = stg1[:].bitcast(bf16)  # [P, 2, DH] view of the fp8 stage
            for f in range(NFILL):
                src = stgb[:, f % 2, (f % 4) * P:(f % 4 + 1) * P]
                nc.tensor.matmul(
                    ftile[:], src, ident[:], is_transpose=True,
                    start=True, stop=True,
                )

        with tc.tile_pool(name="psB", bufs=4, space="PSUM") as psB:
            pa = const.tile([P, 2, D], f8, name="pa")
            pb = const.tile([P, 2, D], f8, name="pb")
            nc.sync.dma_start(pa[:], ar_out[0].rearrange("(c p) d -> p c d", p=P))
            nc.sync.dma_start(pb[:], ar_out[1].rearrange("(c p) d -> p c d", p=P))
            # combine per D-half on separate engines so h0 unblocks early
            ptxb = [const.tile([P, 2, DH], f8, name=f"ptxb{h}") for h in range(2)]
            nc.vector.tensor_add(ptxb[0][:], pa[:, :, 0:DH], pb[:, :, 0:DH])
            nc.gpsimd.tensor_add(ptxb[1][:], pa[:, :, DH:D], pb[:, :, DH:D])

            for i in range(NT):
                cor = psB.tile([P, 2, DH], f32, name="cor", tag="cor")
                for h in range(2):
                    nc.tensor.matmul(
                        cor[:, h, :], Pt[:, :, i * P:(i + 1) * P], ptxb[h][:],
                        perf_mode=DR, start=True, stop=True,
                    )
                # single ACT drain frees the PSUM banks quickly; the
                # gamma-scaled residual adds then run in SBUF.
                cs = cpool.tile([P, 2, DH], f32, name="cs", tag="cs")
                nc.scalar.copy(cs[:], cor[:])
                o_sb = opool.tile([P, 2, DH], f32, name="o_sb", tag="o")
                nc.vector.scalar_tensor_tensor(
                    o_sb[:], cs[:], GAMMA,
                    Xall[:, i, :].rearrange("p (h d) -> p h d", h=2),
                    mybir.AluOpType.mult, mybir.AluOpType.add,
                )
                nc.sync.dma_start(
                    out[i * P:(i + 1) * P, :].rearrange("p (h d) -> p h d", h=2),
                    o_sb[:],
                )

    nc.finalize()
    return nc


def _run(inputs, trace=False, **kwargs):
    import ml_dtypes
    from concourse import bass_utils

    if "nc" not in _cache:
        _cache["nc"] = _build_nc()
    nc = _cache["nc"]

    X = np.asarray(inputs["X"], dtype=np.float32)
    W = np.ascontiguousarray(np.asarray(inputs["W"], dtype=np.float32))
    b = np.ascontiguousarray(np.asarray(inputs["b"], dtype=np.float32))
    Xbf = np.ascontiguousarray(X.astype(ml_dtypes.bfloat16))
    Wt = np.ascontiguousarray(W.T)

    in_maps = [
        {"Xbf": Xbf[i * NLOC:(i + 1) * NLOC], "Wt": Wt, "b": b}
        for i in range(NCORES)
    ]
    res = bass_utils.run_bass_kernel_spmd(
        nc, in_maps, core_ids=list(range(NCORES)), trace=trace, **kwargs
    )
    outp = np.concatenate([res.results[i]["out"] for i in range(NCORES)], axis=0)
    return outp, res


def kernel(**inputs):
    outp, _ = _run(inputs, trace=False)
    return outp


# revision 16
# speedup vs baseline: 1.2092x; 1.2092x over previous
"""Trainium2 Bass kernel for nn_Compression.

Computes: out = X + GAMMA * (P @ (P.T @ X)),  P = softmax(X @ W.T + b)

Strategy (8 NeuronCores, data-parallel over N):
  - Each core owns NLOC = N/8 = 4096 rows of X (32 tiles of 128 rows).
  - X ships to the device as bf16 (host-side cast, pure relayout of the
    shard): the residual add only needs bf16 precision (~1.7e-3
    relative, vs the 2e-2 gate) and this halves the input DMA.
  - fp8e4 in the correction path: the correction term is ~1e-5 of the
    output magnitude, so fp8 quantization (~6% of the correction)
    contributes ~1e-6 relative.  fp8 enables DoubleRow matmuls (two
    contraction rows per cycle) which halve the logits / corr PE time.
    Transposes stay bf16 (fp8 PE transposes need strided outputs) and
    cast to fp8 at the PSUM drain.
  - Phase A per row-tile: PE-transpose of the bf16 tile (drained to
    fp8), logits via 4 DoubleRow matmuls over d-chunk pairs against
    16*W in fp8 (the 1/16 un-scale rides the exp activation's scale),
    softmax on ScalarE.  P.T @ X accumulates into 4 resident PSUM
    banks in bf16 (an fp8 X copy costs more in casts than DoubleRow
    saves on the PE).
  - The PtX accumulation is split into two row-groups sharing the same
    PSUM banks.  Group A's partial is drained to fp8 and AllReduced
    *while phase A keeps running* on the second group, hiding that
    collective (the early SPLIT keeps it clear of the second trigger,
    since collectives serialize on one stream); only group B's
    AllReduce (256 KiB fp8) is exposed at the end, and filler PE
    transposes reading the group-B stage (so the scheduler cannot
    hoist them into phase A) keep the HAM clock-gate warm across it.
  - Phase B: ptxb = arA + arB (fp8), corr = P @ ptxb as one DoubleRow
    matmul per D-half, then out = GAMMA*corr + X as a fused
    scalar_tensor_tensor on DVE / GpSimd, one 512 KiB DMA per tile.
"""

import sys

import numpy as np

if "/opt/trn_rl_repo" not in sys.path:
    sys.path.insert(0, "/opt/trn_rl_repo")

N, D, C = 32768, 1024, 256
GAMMA = 1e-4
WSCALE = 16.0  # keeps 16*W inside fp8e4's normal range
NCORES = 8
NLOC = N // NCORES  # 4096
P = 128
NT = NLOC // P  # 32
DH = 512
SPLIT = 10  # row-tiles in AllReduce group A (triggered early enough that
            # the contended ~36us collective clears phase A's end, so the
            # exposed group-B AllReduce isn't stream-serialized behind it)
NFILL = 600  # PE warm-keeper transposes across the exposed AllReduce

_cache = {}


def _build_nc():
    import concourse.tile as tile
    from concourse import bacc
    import concourse.mybir as mybir
    from concourse.masks import make_identity
    from contextlib import ExitStack

    f32 = mybir.dt.float32
    bf16 = mybir.dt.bfloat16
    f8 = mybir.dt.float8e4
    AF = mybir.ActivationFunctionType
    DR = mybir.MatmulPerfMode.DoubleRow

    nc = bacc.Bacc("TRN2", target_bir_lowering=False, debug=False, num_devices=NCORES)
    Xbf = nc.dram_tensor("Xbf", [NLOC, D], bf16, kind="ExternalInput").ap()
    Wt = nc.dram_tensor("Wt", [D, C], f32, kind="ExternalInput").ap()
    bvec = nc.dram_tensor("b", [C], f32, kind="ExternalInput").ap()
    out = nc.dram_tensor("out", [NLOC, D], f32, kind="ExternalOutput").ap()

    with tile.TileContext(nc) as tc, ExitStack() as ctx:
        const = ctx.enter_context(tc.tile_pool(name="const", bufs=1))
        xres = ctx.enter_context(tc.tile_pool(name="xres", bufs=1))
        work = ctx.enter_context(tc.tile_pool(name="work", bufs=2))
        ppool = ctx.enter_context(tc.tile_pool(name="ppool", bufs=4))
        spool = ctx.enter_context(tc.tile_pool(name="spool", bufs=4))
        stgp = ctx.enter_context(tc.tile_pool(name="stgp", bufs=2))
        opool = ctx.enter_context(tc.tile_pool(name="opool", bufs=4))
        cpool = ctx.enter_context(tc.tile_pool(name="cpool", bufs=4))
        dram = ctx.enter_context(tc.tile_pool(name="dram", bufs=1, space="DRAM"))

        Xall = xres.tile([P, NT, D], bf16)  # residual + matmul operand
        Pt = const.tile([P, 2, NLOC], f8)  # P.T resident (unscaled)

        # X tiles 0-2 first so tile 0's transpose isn't behind the W-chunk
        # DMAs in the queue.
        def s_load(i):
            nc.sync.dma_start(Xall[:, i, :], Xbf[i * P:(i + 1) * P, :])

        ident = const.tile([P, P], bf16)
        make_identity(nc, ident)

        # 16*W.T in fp8, [d-within-chunk, k-chunk, c]; 4 DMA chunks (the
        # first two ahead of the X prefetch so logits(0) isn't starved),
        # cast on ScalarE.
        Wt8 = const.tile([P, 8, C], f8)
        wtmp = ctx.enter_context(tc.tile_pool(name="wtmp", bufs=1))
        wt_f = wtmp.tile([P, 8, C], f32)
        wt_r = Wt.rearrange("(k p) c -> p k c", p=P)

        def load_w(q):
            nc.sync.dma_start(wt_f[:, 2 * q:2 * q + 2, :], wt_r[:, 2 * q:2 * q + 2, :])
            nc.scalar.mul(Wt8[:, 2 * q:2 * q + 2, :], wt_f[:, 2 * q:2 * q + 2, :], WSCALE)

        load_w(0)
        load_w(1)
        for i in range(3):
            s_load(i)
        load_w(2)
        load_w(3)

        ones1 = const.tile([1, P], bf16)
        nc.vector.memset(ones1[:], 1.0)
        b_sb = const.tile([1, C], bf16)  # 16*b
        with tc.tile_pool(name="btmp", bufs=1) as btmp:
            b_f = btmp.tile([1, C], f32)
            nc.sync.dma_start(b_f[:], bvec.rearrange("(o c) -> o c", o=1))
            nc.vector.tensor_scalar_mul(b_sb[:], b_f[:], WSCALE)

        # Two AllReduces (fp8e4, unscaled partials): group A hidden under
        # phase A's second half, group B exposed at the end.
        ar_in = [dram.tile([C, D], f8, name=f"ar_in{g}") for g in range(2)]
        ar_out = [
            dram.tile([C, D], f8, addr_space="Shared", name=f"ar_out{g}")
            for g in range(2)
        ]

        # ---- phase A: software-pipelined over row-tiles ----
        def s_transpose(i):
            xt = work.tile([P, D], f8, name="xt", tag="xt")
            trp = psA.tile([P, D], bf16, name="trp", tag="trp")
            for k in range(8):
                nc.tensor.matmul(
                    trp[:, k * P:(k + 1) * P],
                    Xall[:, i, k * P:(k + 1) * P],
                    ident[:],
                    is_transpose=True,
                    start=(k == 0),
                    stop=(k == 7),
                )
            # drain + fp8 cast, split across ACT and DVE
            nc.scalar.copy(xt[:, 0:DH], trp[:, 0:DH])
            nc.vector.tensor_copy(xt[:, DH:D], trp[:, DH:D])
            return xt

        def s_logits(i, xt):
            lg = psL.tile([P, C], f32, name="lg", tag="lg")
            xt_r = xt[:].rearrange("p (k n) -> p k n", k=8)
            for kk in range(4):
                nc.tensor.matmul(
                    lg[:],
                    xt_r[:, 2 * kk:2 * kk + 2, :],
                    Wt8[:, 2 * kk:2 * kk + 2, :],
                    perf_mode=DR,
                    start=(kk == 0),
                    stop=False,
                )
            nc.tensor.matmul(lg[:], ones1[:], b_sb[:], start=False, stop=True)
            return lg

        def s_softmax(i, lg):
            # logits arrive scaled by 16 (W,b pre-scaled); the 1/16 rides
            # the exp activation's scale.  |logits| <= ~10 so exp is safe
            # without max-subtraction.
            p_sb = ppool.tile([P, C], f32, name="p_sb", tag="p")
            ssum = spool.tile([P, 1], f32, name="ssum", tag="s")
            nc.scalar.activation(p_sb[:], lg[:], AF.Exp, scale=1.0 / WSCALE,
                                 accum_out=ssum[:])
            rinv = spool.tile([P, 1], f32, name="rinv", tag="r")
            nc.vector.reciprocal(rinv[:], ssum[:])
            # normalized P in bf16 (feeds both the PtX matmul and the P.T
            # transpose); fp8 would need an extra cast engine pass and the
            # DoubleRow PtX gain is eaten by it.
            p_bf = ppool.tile([P, C], bf16, name="p_bf", tag="pb")
            nc.scalar.activation(p_bf[:], p_sb[:], AF.Copy, scale=rinv[:])
            return p_bf

        def s_ptx(i, p_bf):
            first = i in (0, SPLIT)
            last = i in (SPLIT - 1, NT - 1)
            for c in range(2):
                for h in range(2):
                    nc.tensor.matmul(
                        ptx_ps[2 * c + h][:],
                        p_bf[:, c * P:(c + 1) * P],
                        Xall[:, i, h * DH:(h + 1) * DH],
                        start=first,
                        stop=last,
                    )
            # P.T for phase B (bf16 transpose, fp8 cast at the DVE drain)
            ptp = psA.tile([P, C], bf16, name="ptp", tag="trp")
            for c in range(2):
                nc.tensor.matmul(
                    ptp[:, c * P:(c + 1) * P],
                    p_bf[:, c * P:(c + 1) * P],
                    ident[:],
                    is_transpose=True,
                    start=(c == 0),
                    stop=(c == 1),
                )
            nc.vector.tensor_copy(
                Pt[:, :, i * P:(i + 1) * P],
                ptp[:].rearrange("p (c n) -> p c n", c=2),
            )

        def drain_and_reduce(g):
            # PSUM -> SBUF fp8 casts (split across ACT and DVE), each
            # quadrant DMA'd to DRAM as soon as its cast lands, then the
            # AllReduce trigger.
            stg = stgp.tile([P, 2, D], f8, name=f"stg{g}", tag="stg")
            ar_v = ar_in[g].rearrange("(c p) d -> p c d", p=P)
            for c in range(2):
                for h in range(2):
                    dst = stg[:, c, h * DH:(h + 1) * DH]
                    src = ptx_ps[2 * c + h]
                    if h == 0:
                        nc.scalar.copy(dst, src[:])
                    else:
                        nc.vector.tensor_copy(dst, src[:])
                    nc.sync.dma_start(ar_v[:, c, h * DH:(h + 1) * DH], dst)
            nc.gpsimd.collective_compute(
                "AllReduce",
                mybir.AluOpType.add,
                replica_groups=[list(range(NCORES))],
                ins=[ar_in[g][:].opt()],
                outs=[ar_out[g][:].opt()],
            )
            return stg

        with tc.tile_pool(name="psA", bufs=3, space="PSUM") as psA, \
             tc.tile_pool(name="psL", bufs=1, space="PSUM") as psL, \
             tc.tile_pool(name="psX", bufs=1, space="PSUM") as psX:
            ptx_ps = [
                psX.tile([P, DH], f32, name=f"ptx_{c}_{h}", tag=f"ptx_{c}_{h}")
                for c in range(2)
                for h in range(2)
            ]
            # 2-3 tile skew between softmax(i) and the pair PtX: the ~1us
            # ScalarE exp latency hides under transposes + the next logits
            # block instead of stalling the PE.
            xts = {0: s_transpose(0)}
            pbf = {}
            for i in range(NT):
                lg = s_logits(i, xts.pop(i))
                pbf[i] = s_softmax(i, lg)
                if i >= 2:
                    s_ptx(i - 2, pbf.pop(i - 2))
                    if i - 2 == SPLIT - 1:
                        drain_and_reduce(0)
                if i + 1 < NT:
                    xts[i + 1] = s_transpose(i + 1)
                if i + 2 < NT:
                    s_load(i + 2)
            for i in (NT - 2, NT - 1):
                s_ptx(i, pbf.pop(i))
            stg1 = drain_and_reduce(1)

        # ---- exposed-collective window + phase B ----
        # Keep the HAM clock-gate warm while AllReduce B flies.  The
        # fillers read the group-B stage so the scheduler cannot hoist
        # them before the end of phase A.
        with tc.tile_pool(name="psF", bufs=1, space="PSUM") as psF:
            ftile = psF.tile([P, P], bf16, name="fill", tag="fill")
            stgb = stg1[:].bitcast(bf16)  # [P, 2, DH] view of the fp8 stage
            for f in range(NFILL):
                src = stgb[:, f % 2, (f % 4) * P:(f % 4 + 1) * P]
                nc.tensor.matmul(
                    ftile[:], src, ident[:], is_transpose=True,
                    start=True, stop=True,
                )

        with tc.tile_pool(name="psB", bufs=4, space="PSUM") as psB:
            pa = const.tile([P, 2, D], f8, name="pa")
            pb = const.tile([P, 2, D], f8, name="pb")
            nc.sync.dma_start(pa[:], ar_out[0].rearrange("(c p) d -> p c d", p=P))
            nc.sync.dma_start(pb[:], ar_out[1].rearrange("(c p) d -> p c d", p=P))
            # combine per D-half on separate engines so h0 unblocks early
            ptxb = [const.tile([P, 2, DH], f8, name=f"ptxb{h}") for h in range(2)]
            nc.vector.tensor_add(ptxb[0][:], pa[:, :, 0:DH], pb[:, :, 0:DH])
            nc.gpsimd.tensor_add(ptxb[1][:], pa[:, :, DH:D], pb[:, :, DH:D])

            for i in range(NT):
                cor = psB.tile([P, 2, DH], f32, name="cor", tag="cor")
                for h in range(2):
                    nc.tensor.matmul(
                        cor[:, h, :], Pt[:, :, i * P:(i + 1) * P], ptxb[h][:],
                        perf_mode=DR, start=True, stop=True,
                    )
                # single ACT drain frees the PSUM banks quickly; the
                # gamma-scaled residual adds then run in SBUF.
                cs = cpool.tile([P, 2, DH], f32, name="cs", tag="cs")
                nc.scalar.copy(cs[:], cor[:])
                o_sb = opool.tile([P, 2, DH], f32, name="o_sb", tag="o")
                nc.vector.scalar_tensor_tensor(
                    o_sb[:], cs[:], GAMMA,
                    Xall[:, i, :].rearrange("p (h d) -> p h d", h=2),
                    mybir.AluOpType.mult, mybir.AluOpType.add,
                )
                nc.sync.dma_start(
                    out[i * P:(i + 1) * P, :].rearrange("p (h d) -> p h d", h=2),
                    o_sb[:],
                )

    nc.finalize()
    return nc


def _run(inputs, trace=False, **kwargs):
    import ml_dtypes
    from concourse import bass_utils

    if "nc" not in _cache:
        _cache["nc"] = _build_nc()
    nc = _cache["nc"]

    X = np.asarray(inputs["X"], dtype=np.float32)
    W = np.ascontiguousarray(np.asarray(inputs["W"], dtype=np.float32))
    b = np.ascontiguousarray(np.asarray(inputs["b"], dtype=np.float32))
    Xbf = np.ascontiguousarray(X.astype(ml_dtypes.bfloat16))
    Wt = np.ascontiguousarray(W.T)

    in_maps = [
        {"Xbf": Xbf[i * NLOC:(i + 1) * NLOC], "Wt": Wt, "b": b}
        for i in range(NCORES)
    ]
    res = bass_utils.run_bass_kernel_spmd(
        nc, in_maps, core_ids=list(range(NCORES)), trace=trace, **kwargs
    )
    outp = np.concatenate([res.results[i]["out"] for i in range(NCORES)], axis=0)
    return outp, res


def kernel(**inputs):
    outp, _ = _run(inputs, trace=False)
    return outp
